# revision 15
# baseline (speedup 1.0000x reference)
"""Trainium2 Bass kernel for nn_BandwidthPredictorNNHall.

Math: for each batch b (8 of them, one per NeuronCore) with particles
x [n=1024, d=4]:
    pilot_d = 1.0592 * std(x_d, ddof=1) * n^(-1/8)
    q = x / pilot,   K_ij = exp(-0.5 * |q_i - q_j|^2)
    s2_d = sum_ij K_ij ((q_jd - q_id)^2 - 1)
    s3 terms are exactly 0 by antisymmetry (treated as 0; fp noise in the
    reference, |bw2/bw1| ~ 6e-9).
With Mp = [1, p_1..p_4, p_1^2..p_4^2] (n x 9, RAW particle units), every sum
needed for s2 is an entry of V = Mp^T K Mp:
    s2_d = ((V[0,5+d] + V[5+d,0] - 2 V[1+d,1+d]) / pilot_d^2 - V[0,0]) / sqrt(2pi)

K-symmetry splits V = Vu + Vu^T - Vd where Vu sums the upper block
triangle (128-row blocks, diagonal blocks whole) and Vd the diagonal
blocks only.  The device computes, per 128-column chunk c, the [128, 9]
partial sums
    S_c[p, m] = sum_{i in upper tiles} K''_{i, j(c,p)} Mp[i, m]
    D_c[p, m] = same but diagonal tile only
where K'' = exp(G - r_i/2) = K * e^{+r_j/2} (the column scale is removed
on the host), and ships the raw [128, 16, 9] block plus var to the host,
which applies MX = Mp * e^{-r/2} and the remaining ~10k-flop reduction in
f64.

Device pipeline per core (latency-driven; ScalarE's exps are the floor):
  - One input DMA in a partition-contiguous layout (partition p holds the
    8 consecutive particles 8p..8p+7 as "tiles" c=0..7: 128 descriptors of
    128B).  All downstream math is permutation-invariant under the
    enumeration j = c*128 + p used consistently on both sides of K.
  - Stats run on the PE in BOTH orientations (row [1,4] for the exp-bias
    broadcast chain, col [4,1] for the per-partition Q scaling) so neither
    needs a transpose of the other.
  - QTr (feature-major f32r) via 8 PE transposes packed as two [4,512]
    PSUM quads (first transpose per quad start=True marks the 2KB
    zero-region, the rest start=False zero-on-touch), drained by two
    512-wide ScalarE copies while DVE runs the var chains.
  - K'' row tiles, UPPER TRIANGLE ONLY: tile ir covers columns
    j >= 128*ir (4608 exp columns instead of 8192), one exp per tile with
    per-partition bias -r_i/2.
  - P-stage contracts over i on the PE with K'' chunks as the stationary
    operand: accT[:, 2jb, :] += KT_chunk(lhsT) @ Mp_tile (~15ns each:
    9-column outputs; weight loads are free), diag-only twin at 2jb+1.
    start=False everywhere + one upfront memset: a start=True would mark
    the whole shared 2KB PSUM bank pending-zero and wipe the co-resident
    accumulators.
  - Chunk c's accumulation finishes at iteration c: one [128,2,9] DVE
    copy per chunk drains it to SBUF in-stream.  After the last tile only
    chunk 7's drain remains, then a single output DMA.
"""

import sys

sys.path.insert(0, "/opt/trn_rl_repo")

import numpy as np

_B, _N, _D = 8, 1024, 4
_P = 128
_NT = _N // _P  # 8 column/row tiles
_NM = 1 + 2 * _D  # 9 basis columns: [1, p, p^2]
_INV_SQRT_2PI = 1.0 / np.sqrt(2.0 * np.pi)
_RK = 0.282095
_FACT = 1.0592 * float(_N) ** (-1.0 / (4 + _D))

_NC = None  # compiled Bass module cache


def _build_kernel():
    import concourse.bass as bass  # noqa: F401
    import concourse.tile as tile
    from concourse import bacc, mybir
    from concourse.masks import make_identity

    f32 = mybir.dt.float32
    fr = mybir.dt.float32r
    Act = mybir.ActivationFunctionType
    Alu = mybir.AluOpType
    Ax = mybir.AxisListType

    nc = bacc.Bacc("TRN2", target_bir_lowering=False, debug=False, num_devices=_B)
    p_in = nc.dram_tensor("p", [_N, _D], f32, kind="ExternalInput")
    v_out = nc.dram_tensor("vout", [_P, 16 * _NM], f32, kind="ExternalOutput")
    var_out = nc.dram_tensor("varout", [1, _D], f32, kind="ExternalOutput")

    with tile.TileContext(nc) as tc:
        with (
            tc.tile_pool(name="singles", bufs=1) as singles,
            tc.tile_pool(name="psE", bufs=1, space="PSUM") as psE,
            tc.tile_pool(name="psA", bufs=1, space="PSUM") as psA,
            tc.tile_pool(name="psG", bufs=3, space="PSUM") as psG,
        ):
            # ---- input DMA first in SP program order (data-ready gates all)
            mstatall = singles.tile([_P, _NT, _D], f32, tag="mstatall")
            nc.sync.dma_start(
                out=mstatall, in_=p_in[:].rearrange("(i c) d -> i c d", i=_P)
            )

            # dummy Exp so the activation-table load runs during the DMA wait
            warm = singles.tile([1, 1], f32, tag="warm")

            ident128 = singles.tile([_P, _P], f32, tag="identf")
            make_identity(nc, ident128)
            ones128 = singles.tile([_P, 1], f32, tag="ones128")
            nc.gpsimd.memset(ones128, 1.0)
            onesN = singles.tile([_P, 1], f32, tag="onesN")
            nc.gpsimd.memset(onesN, 1.0 / float(_N) ** 0.5)
            nc.scalar.activation(out=warm, in_=ident128[0:1, 0:1], func=Act.Exp)

            msqall = singles.tile([_P, _NT, _D], f32, tag="msqall")
            nc.vector.tensor_mul(msqall, mstatall, mstatall)

            # ---- stats on the PE, both orientations, one PSUM bank:
            #  row sums at early[0:1, 4:8] (p) and [0:1, 8:12] (p^2)
            #  col sums at early[0:4, 12:13] (p) and [0:4, 13:14] (p^2)
            early = psE.tile([_P, 16], f32, tag="early")
            for c in range(_NT):
                nc.tensor.matmul(
                    early[0:1, 4:8], lhsT=onesN, rhs=mstatall[:, c, :],
                    start=(c == 0), stop=(c == _NT - 1), skip_group_check=True,
                )
            for c in range(_NT):
                nc.tensor.matmul(
                    early[0:4, 12:13], lhsT=mstatall[:, c, :], rhs=onesN,
                    start=(c == 0), stop=(c == _NT - 1), skip_group_check=True,
                )
            for c in range(_NT):
                nc.tensor.matmul(
                    early[0:1, 8:12], lhsT=ones128, rhs=msqall[:, c, :],
                    start=(c == 0), stop=(c == _NT - 1), skip_group_check=True,
                )
            for c in range(_NT):
                nc.tensor.matmul(
                    early[0:4, 13:14], lhsT=msqall[:, c, :], rhs=ones128,
                    start=(c == 0), stop=(c == _NT - 1), skip_group_check=True,
                )

            # ---- 8 PE transposes -> QTr, packed as two [4,512] quads in
            # rotating psG slots (dead before Gram needs the slots back),
            # each drained by one 512-wide ScalarE copy
            QTr = singles.tile([_D, _N], fr, tag="qtr")
            for q in range(2):
                ps_q = psG.tile([_D, 4 * _P], f32, tag="psg", name=f"psq{q}")
                for k in range(4):
                    c = q * 4 + k
                    nc.tensor.matmul(
                        ps_q[:, k * _P : (k + 1) * _P],
                        lhsT=mstatall[:, c, :], rhs=ident128,
                        is_transpose=True, start=(k == 0), stop=True,
                        skip_group_check=True,
                    )
                cs = slice(q * 4 * _P, (q + 1) * 4 * _P)
                nc.scalar.activation(out=QTr[:, cs], in_=ps_q, func=Act.Copy)

            # ---- var chains on DVE (col form first: it gates the Gram
            # lhsT scale); stats copies from PSUM on DVE
            svr = singles.tile([1, 8], f32, tag="svr")
            nc.vector.tensor_copy(svr, early[0:1, 4:12])
            svc = singles.tile([_D, 2], f32, tag="svc")
            nc.vector.tensor_copy(svc, early[0:4, 12:14])

            den_c = singles.tile([_D, 1], f32, tag="den_c")
            nc.vector.tensor_mul(den_c, svc[:, 0:1], svc[:, 0:1])
            nc.vector.tensor_sub(den_c, svc[:, 1:2], den_c)
            denf_c = singles.tile([_D, 1], f32, tag="denf_c")
            nc.vector.tensor_scalar_mul(denf_c, den_c, _FACT * _FACT / (_N - 1))
            phcol = singles.tile([_D, 1], f32, tag="phcol")
            nc.vector.reciprocal(phcol, denf_c)  # 1/pilot^2

            den_r = singles.tile([1, _D], f32, tag="den_r")
            nc.vector.tensor_mul(den_r, svr[:, 0:4], svr[:, 0:4])
            nc.vector.tensor_sub(den_r, svr[:, 4:8], den_r)
            denf_r = singles.tile([1, _D], f32, tag="denf_r")
            nc.vector.tensor_scalar_mul(denf_r, den_r, 2.0 * _FACT * _FACT / (_N - 1))
            ph_row = singles.tile([1, _D], f32, tag="ph_row")
            nc.vector.reciprocal(ph_row, denf_r)  # 0.5/pilot^2
            var_t = singles.tile([1, _D], f32, tag="var_t")
            nc.gpsimd.tensor_scalar_mul(var_t, den_r, 1.0 / (_N - 1))
            nc.sync.dma_start(out=var_out[:], in_=var_t)

            # bc_sb[128, 4] = ph_row broadcast to all partitions (gpsimd)
            bc_sb = singles.tile([_P, _D], f32, tag="bc_sb")
            nc.gpsimd.partition_broadcast(bc_sb, ph_row)

            # ---- exp-bias chain and Gram lhsT scales, interleaved so
            # tile 0 completes first: scr/qs on gpsimd, reduces on DVE
            qs_t = singles.tile([_D, _NT, _P], fr, tag="qs_t")
            nhall = singles.tile([_P, _NT], f32, tag="nhall")
            scr = singles.tile([_P, _NT, _D], f32, tag="scr")
            for c in range(_NT):
                nc.gpsimd.tensor_mul(scr[:, c, :], msqall[:, c, :], bc_sb)
            for c in range(_NT):
                cs = slice(c * _P, (c + 1) * _P)
                nc.vector.tensor_scalar_mul(qs_t[:, c, :], QTr[:, cs], phcol)
                nc.vector.tensor_reduce(
                    out=nhall[:, c : c + 1], in_=scr[:, c, :],
                    axis=Ax.X, op=Alu.add, negate=True,
                )

            # ---- Mp tiles (f32): [1, p, p^2]
            mtall = singles.tile([_P, _NT, _NM], f32, tag="mtall")
            for c in range(_NT):
                nc.gpsimd.tensor_copy(mtall[:, c, 0:1], ones128)
            nc.gpsimd.tensor_copy(mtall[:, :, 1 : 1 + _D], mstatall)
            nc.gpsimd.tensor_copy(mtall[:, :, 1 + _D : _NM], msqall)

            # ---- PSUM accumulators, one bank: chunk c's upper-sum at
            # [:, 2c, :], diag-only twin at [:, 2c+1, :]
            accT = psA.tile([_P, 16, _NM], f32, tag="accT")
            # start=True on any matmul would mark the whole 2KB zero-region
            # pending-zero and wipe co-resident accumulators: zero once,
            # accumulate-only afterwards
            nc.vector.memset(accT, 0.0)

            KT = singles.tile([_P, _NT, _N], f32, tag="kt")
            Psb = singles.tile([_P, 16, _NM], f32, tag="psb")

            def emit_gram(ir):
                psg = psG.tile([_P, _N], f32, tag="psg", name=f"psg{ir}")
                for a, b in ((0, 512), (512, 1024)):
                    nc.tensor.matmul(
                        psg[:, a:b], lhsT=qs_t[:, ir, :], rhs=QTr[:, a:b],
                        start=True, stop=True,
                    )
                return psg

            def emit_exp(ir, psg):
                if ir == 0:
                    # split: the first half only needs the first QTr quad,
                    # so it starts before the second quad-copy lands
                    nc.scalar.activation(
                        out=KT[:, 0, 0:512], in_=psg[:, 0:512],
                        func=Act.Exp, bias=nhall[:, 0:1],
                    )
                    nc.scalar.activation(
                        out=KT[:, 0, 512:_N], in_=psg[:, 512:_N],
                        func=Act.Exp, bias=nhall[:, 0:1],
                    )
                    return
                s = ir * _P
                nc.scalar.activation(
                    out=KT[:, ir, s:_N], in_=psg[:, s:_N],
                    func=Act.Exp, bias=nhall[:, ir : ir + 1],
                )

            def emit_pdirect(ir):
                for jb in range(ir, _NT):
                    nc.tensor.matmul(
                        accT[:, 2 * jb, :],
                        lhsT=KT[:, ir, jb * _P : (jb + 1) * _P],
                        rhs=mtall[:, ir, :],
                        start=False, stop=(ir == jb),
                        skip_group_check=True,
                    )
                nc.tensor.matmul(
                    accT[:, 2 * ir + 1, :],
                    lhsT=KT[:, ir, ir * _P : (ir + 1) * _P],
                    rhs=mtall[:, ir, :],
                    start=False, stop=True,
                    skip_group_check=True,
                )

            def emit_drain(c):
                if c == _NT - 1:
                    # last chunk drains on ScalarE (idle after the exps),
                    # in parallel with DVE's chunk-6 drain
                    nc.scalar.activation(
                        out=Psb[:, 2 * c : 2 * c + 2, :],
                        in_=accT[:, 2 * c : 2 * c + 2, :], func=Act.Copy,
                    )
                    return
                nc.vector.tensor_copy(
                    Psb[:, 2 * c : 2 * c + 2, :], accT[:, 2 * c : 2 * c + 2, :]
                )

            # ---- main triangle loop: Grams three ahead (psG bufs=3
            # absorbs the exp->WAR latency), P-stage and drains one behind
            grams = [emit_gram(0), emit_gram(1), emit_gram(2)]
            for ir in range(_NT):
                emit_exp(ir, grams[ir])
                if ir + 3 < _NT:
                    grams.append(emit_gram(ir + 3))
                if ir >= 1:
                    emit_pdirect(ir - 1)
                    emit_drain(ir - 1)
            vr = v_out[:].rearrange("p (a b) -> p a b", a=16)
            nc.sync.dma_start(out=vr[:, 0:12, :], in_=Psb[:, 0:12, :])
            emit_pdirect(_NT - 1)
            emit_drain(_NT - 1)
            nc.sync.dma_start(out=vr[:, 12:16, :], in_=Psb[:, 12:16, :])

    nc.compile()
    return nc


def _get_nc():
    global _NC
    if _NC is None:
        _NC = _build_kernel()
    return _NC


def finalize(raw, var, particles):
    """Host-side tail in f64: raw [128, 144] = per-chunk [S_c | D_c]
    pairs (column-scaled by e^{+r_j/2}), var [4], particles [1024, 4]
    -> bandwidth [4]."""
    raw = raw.astype(np.float64).reshape(_P, 16, _NM)
    var = var.astype(np.float64).reshape(_D)
    p = particles.astype(np.float64)
    pilot2 = (_FACT * _FACT) * var

    Vu = np.zeros((_NM, _NM))
    Vd = np.zeros((_NM, _NM))
    for c in range(_NT):
        # chunk c, partition q <-> particle 8q + c
        pc = p[c::_NT]  # [128, 4]
        Mp = np.concatenate([np.ones((_P, 1)), pc, pc * pc], axis=1)
        r = (pc * pc / pilot2).sum(axis=1)
        MX = Mp * np.exp(-0.5 * r)[:, None]
        Vu += MX.T @ raw[:, 2 * c, :]
        Vd += MX.T @ raw[:, 2 * c + 1, :]
    V = Vu + Vu.T - Vd

    d = np.arange(_D)
    s2 = (
        (V[0, 5 + d] + V[5 + d, 0] - 2.0 * V[1 + d, 1 + d]) / pilot2 - V[0, 0]
    ) * _INV_SQRT_2PI
    denom = _N * (_N - 1)
    I2 = s2 / pilot2**2.5 / denom
    J1 = _RK / I2
    base = J1 / _N
    return (np.sign(base) * np.abs(base) ** 0.2).astype(np.float32)


def kernel(particles, weights=None, **_unused):
    from concourse.bass_utils import run_bass_kernel_spmd

    particles = np.ascontiguousarray(np.asarray(particles), dtype=np.float32)
    assert particles.shape == (_B, _N, _D), particles.shape

    nc = _get_nc()
    in_maps = [{"p": particles[c]} for c in range(_B)]
    res = run_bass_kernel_spmd(nc, in_maps, list(range(_B)))

    out = np.empty((_B, _D), np.float32)
    for c in range(_B):
        out[c] = finalize(
            res.results[c]["vout"], res.results[c]["varout"], particles[c]
        )
    return out


# revision 16
# speedup vs baseline: 1.0512x; 1.0512x over previous
"""Trainium2 Bass kernel for nn_BandwidthPredictorNNHall.

Math: for each batch b (8 of them, one per NeuronCore) with particles
x [n=1024, d=4]:
    pilot_d = 1.0592 * std(x_d, ddof=1) * n^(-1/8)
    q = x / pilot,   K_ij = exp(-0.5 * |q_i - q_j|^2)
    s2_d = sum_ij K_ij ((q_jd - q_id)^2 - 1)
    s3 terms are exactly 0 by antisymmetry (treated as 0; fp noise in the
    reference, |bw2/bw1| ~ 6e-9).
With Mp = [1, p_1..p_4, p_1^2..p_4^2] (n x 9, RAW particle units), every sum
needed for s2 is an entry of V = Mp^T K Mp:
    s2_d = ((V[0,5+d] + V[5+d,0] - 2 V[1+d,1+d]) / pilot_d^2 - V[0,0]) / sqrt(2pi)

K-symmetry splits V = Vu + Vu^T - Vd where Vu sums the upper block
triangle (128-row blocks, diagonal blocks whole) and Vd the diagonal
blocks only.  The device computes, per 128-column chunk c, the [128, 9]
partial sums
    S_c[p, m] = sum_{i in upper tiles} K''_{i, j(c,p)} Mp[i, m]
    D_c[p, m] = same but diagonal tile only
where K'' = exp(G - r_i/2) = K * e^{+r_j/2} (the column scale is removed
on the host), and ships the raw [128, 16, 9] block plus var to the host,
which applies MX = Mp * e^{-r/2} and the remaining ~10k-flop reduction in
f64.

Device pipeline per core (latency-driven; ScalarE's exps are the floor):
  - One input DMA in a partition-contiguous layout (partition p holds the
    8 consecutive particles 8p..8p+7 as "tiles" c=0..7: 128 descriptors of
    128B).  All downstream math is permutation-invariant under the
    enumeration j = c*128 + p used consistently on both sides of K.
  - Stats run on the PE in BOTH orientations (row [1,4] for the exp-bias
    broadcast chain, col [4,1] for the per-partition Q scaling) so neither
    needs a transpose of the other.
  - QTr (feature-major f32r) via 8 PE transposes packed as two [4,512]
    PSUM quads (first transpose per quad start=True marks the 2KB
    zero-region, the rest start=False zero-on-touch), drained by two
    512-wide ScalarE copies while DVE runs the var chains.
  - K'' row tiles, UPPER TRIANGLE ONLY: tile ir covers columns
    j >= 128*ir (4608 exp columns instead of 8192), one exp per tile with
    per-partition bias -r_i/2.
  - P-stage contracts over i on the PE with K'' chunks as the stationary
    operand: accT[:, 2jb, :] += KT_chunk(lhsT) @ Mp_tile (~15ns each:
    9-column outputs; weight loads are free), diag-only twin at 2jb+1.
    start=False everywhere + one upfront memset: a start=True would mark
    the whole shared 2KB PSUM bank pending-zero and wipe the co-resident
    accumulators.
  - Chunk c's accumulation finishes at iteration c: one [128,2,9] DVE
    copy per chunk drains it to SBUF in-stream.  After the last tile only
    chunk 7's drain remains, then a single output DMA.
"""

import sys

sys.path.insert(0, "/opt/trn_rl_repo")

import numpy as np

_B, _N, _D = 8, 1024, 4
_P = 128
_NT = _N // _P  # 8 column/row tiles
_NM = 1 + 2 * _D  # 9 basis columns: [1, p, p^2]
_INV_SQRT_2PI = 1.0 / np.sqrt(2.0 * np.pi)
_RK = 0.282095
_FACT = 1.0592 * float(_N) ** (-1.0 / (4 + _D))

_NC = None  # compiled Bass module cache


def _build_kernel():
    import concourse.bass as bass  # noqa: F401
    import concourse.tile as tile
    from concourse import bacc, mybir
    from concourse.masks import make_identity

    f32 = mybir.dt.float32
    fr = mybir.dt.float32r
    Act = mybir.ActivationFunctionType
    Alu = mybir.AluOpType
    Ax = mybir.AxisListType

    nc = bacc.Bacc("TRN2", target_bir_lowering=False, debug=False, num_devices=_B)
    p_in = nc.dram_tensor("p", [_N, _D], f32, kind="ExternalInput")
    v_out = nc.dram_tensor("vout", [_P, 16 * _NM], f32, kind="ExternalOutput")
    var_out = nc.dram_tensor("varout", [1, _D], f32, kind="ExternalOutput")

    with tile.TileContext(nc) as tc:
        with (
            tc.tile_pool(name="singles", bufs=1) as singles,
            tc.tile_pool(name="psE", bufs=1, space="PSUM") as psE,
            tc.tile_pool(name="psA", bufs=1, space="PSUM") as psA,
            tc.tile_pool(name="psG", bufs=3, space="PSUM") as psG,
        ):
            # ---- input DMA first in SP program order (data-ready gates all)
            mstatall = singles.tile([_P, _NT, _D], f32, tag="mstatall")
            nc.sync.dma_start(
                out=mstatall, in_=p_in[:].rearrange("(i c) d -> i c d", i=_P)
            )

            # dummy Exp so the activation-table load runs during the DMA wait
            warm = singles.tile([1, 1], f32, tag="warm")

            ident128 = singles.tile([_P, _P], f32, tag="identf")
            make_identity(nc, ident128)
            ones128 = singles.tile([_P, 1], f32, tag="ones128")
            nc.gpsimd.memset(ones128, 1.0)
            onesN = singles.tile([_P, 1], f32, tag="onesN")
            nc.gpsimd.memset(onesN, 1.0 / float(_N) ** 0.5)
            nc.scalar.activation(out=warm, in_=ident128[0:1, 0:1], func=Act.Exp)

            msqall = singles.tile([_P, _NT, _D], f32, tag="msqall")
            nc.vector.tensor_mul(msqall, mstatall, mstatall)

            # ---- stats on the PE, both orientations, one PSUM bank:
            #  row sums at early[0:1, 4:8] (p) and [0:1, 8:12] (p^2)
            #  col sums at early[0:4, 12:13] (p) and [0:4, 13:14] (p^2)
            early = psE.tile([_P, 16], f32, tag="early")
            for c in range(_NT):
                nc.tensor.matmul(
                    early[0:1, 4:8], lhsT=onesN, rhs=mstatall[:, c, :],
                    start=(c == 0), stop=(c == _NT - 1), skip_group_check=True,
                )
            for c in range(_NT):
                nc.tensor.matmul(
                    early[0:4, 12:13], lhsT=mstatall[:, c, :], rhs=onesN,
                    start=(c == 0), stop=(c == _NT - 1), skip_group_check=True,
                )
            for c in range(_NT):
                nc.tensor.matmul(
                    early[0:1, 8:12], lhsT=ones128, rhs=msqall[:, c, :],
                    start=(c == 0), stop=(c == _NT - 1), skip_group_check=True,
                )
            for c in range(_NT):
                nc.tensor.matmul(
                    early[0:4, 13:14], lhsT=msqall[:, c, :], rhs=ones128,
                    start=(c == 0), stop=(c == _NT - 1), skip_group_check=True,
                )

            # ---- 8 PE transposes -> QTr, packed as two [4,512] quads in
            # rotating psG slots (dead before Gram needs the slots back),
            # each drained by one 512-wide ScalarE copy
            QTr = singles.tile([_D, _N], fr, tag="qtr")
            for q in range(2):
                ps_q = psG.tile([_D, 4 * _P], f32, tag="psg", name=f"psq{q}")
                for k in range(4):
                    c = q * 4 + k
                    nc.tensor.matmul(
                        ps_q[:, k * _P : (k + 1) * _P],
                        lhsT=mstatall[:, c, :], rhs=ident128,
                        is_transpose=True, start=(k == 0), stop=True,
                        skip_group_check=True,
                    )
                cs = slice(q * 4 * _P, (q + 1) * 4 * _P)
                nc.scalar.activation(out=QTr[:, cs], in_=ps_q, func=Act.Copy)

            # ---- var chains on DVE (col form first: it gates the Gram
            # lhsT scale); stats copies from PSUM on DVE
            svr = singles.tile([1, 8], f32, tag="svr")
            nc.vector.tensor_copy(svr, early[0:1, 4:12])
            svc = singles.tile([_D, 2], f32, tag="svc")
            nc.vector.tensor_copy(svc, early[0:4, 12:14])

            den_c = singles.tile([_D, 1], f32, tag="den_c")
            nc.vector.tensor_mul(den_c, svc[:, 0:1], svc[:, 0:1])
            nc.vector.tensor_sub(den_c, svc[:, 1:2], den_c)
            denf_c = singles.tile([_D, 1], f32, tag="denf_c")
            nc.vector.tensor_scalar_mul(denf_c, den_c, _FACT * _FACT / (_N - 1))
            phcol = singles.tile([_D, 1], f32, tag="phcol")
            nc.vector.reciprocal(phcol, denf_c)  # 1/pilot^2

            den_r = singles.tile([1, _D], f32, tag="den_r")
            nc.vector.tensor_mul(den_r, svr[:, 0:4], svr[:, 0:4])
            nc.vector.tensor_sub(den_r, svr[:, 4:8], den_r)
            denf_r = singles.tile([1, _D], f32, tag="denf_r")
            nc.vector.tensor_scalar_mul(denf_r, den_r, 2.0 * _FACT * _FACT / (_N - 1))
            ph_row = singles.tile([1, _D], f32, tag="ph_row")
            nc.vector.reciprocal(ph_row, denf_r)  # 0.5/pilot^2
            var_t = singles.tile([1, _D], f32, tag="var_t")
            nc.gpsimd.tensor_scalar_mul(var_t, den_r, 1.0 / (_N - 1))
            nc.sync.dma_start(out=var_out[:], in_=var_t)

            # bc_sb[128, 4] = ph_row broadcast to all partitions (gpsimd)
            bc_sb = singles.tile([_P, _D], f32, tag="bc_sb")
            nc.gpsimd.partition_broadcast(bc_sb, ph_row)

            # ---- exp-bias chain and Gram lhsT scales, interleaved so
            # tile 0 completes first: scr/qs on gpsimd, reduces on DVE
            qs_t = singles.tile([_D, _NT, _P], fr, tag="qs_t")
            nhall = singles.tile([_P, _NT], f32, tag="nhall")
            scr = singles.tile([_P, _NT, _D], f32, tag="scr")
            for c in range(_NT):
                nc.gpsimd.tensor_mul(scr[:, c, :], msqall[:, c, :], bc_sb)
            for c in range(_NT):
                cs = slice(c * _P, (c + 1) * _P)
                nc.vector.tensor_scalar_mul(qs_t[:, c, :], QTr[:, cs], phcol)
                nc.vector.tensor_reduce(
                    out=nhall[:, c : c + 1], in_=scr[:, c, :],
                    axis=Ax.X, op=Alu.add, negate=True,
                )

            # ---- Mp tiles (f32): [1, p, p^2]
            mtall = singles.tile([_P, _NT, _NM], f32, tag="mtall")
            for c in range(_NT):
                nc.gpsimd.tensor_copy(mtall[:, c, 0:1], ones128)
            nc.gpsimd.tensor_copy(mtall[:, :, 1 : 1 + _D], mstatall)
            nc.gpsimd.tensor_copy(mtall[:, :, 1 + _D : _NM], msqall)

            # ---- PSUM accumulators, one bank: chunk c's upper-sum at
            # [:, 2c, :], diag-only twin at [:, 2c+1, :]
            accT = psA.tile([_P, 16, _NM], f32, tag="accT")
            # start=True on any matmul would mark the whole 2KB zero-region
            # pending-zero and wipe co-resident accumulators: zero once,
            # accumulate-only afterwards
            nc.vector.memset(accT, 0.0)

            KT = singles.tile([_P, _NT, _N], f32, tag="kt")
            Psb = singles.tile([_P, 16, _NM], f32, tag="psb")

            def emit_gram(ir):
                psg = psG.tile([_P, _N], f32, tag="psg", name=f"psg{ir}")
                for a, b in ((0, 512), (512, 1024)):
                    nc.tensor.matmul(
                        psg[:, a:b], lhsT=qs_t[:, ir, :], rhs=QTr[:, a:b],
                        start=True, stop=True,
                    )
                return psg

            def emit_exp(ir, psg):
                s = ir * _P
                nc.scalar.activation(
                    out=KT[:, ir, s:_N], in_=psg[:, s:_N],
                    func=Act.Exp, bias=nhall[:, ir : ir + 1],
                )

            def emit_pdirect(ir):
                for jb in range(ir, _NT):
                    nc.tensor.matmul(
                        accT[:, 2 * jb, :],
                        lhsT=KT[:, ir, jb * _P : (jb + 1) * _P],
                        rhs=mtall[:, ir, :],
                        start=False, stop=(ir == jb),
                        skip_group_check=True,
                    )
                nc.tensor.matmul(
                    accT[:, 2 * ir + 1, :],
                    lhsT=KT[:, ir, ir * _P : (ir + 1) * _P],
                    rhs=mtall[:, ir, :],
                    start=False, stop=True,
                    skip_group_check=True,
                )

            def emit_drain(c):
                if c == _NT - 1:
                    # last chunk drains on ScalarE (idle after the exps),
                    # in parallel with DVE's chunk-6 drain
                    nc.scalar.activation(
                        out=Psb[:, 2 * c : 2 * c + 2, :],
                        in_=accT[:, 2 * c : 2 * c + 2, :], func=Act.Copy,
                    )
                    return
                nc.vector.tensor_copy(
                    Psb[:, 2 * c : 2 * c + 2, :], accT[:, 2 * c : 2 * c + 2, :]
                )

            # ---- main triangle loop: Grams three ahead (psG bufs=3
            # absorbs the exp->WAR latency), P-stage and drains one behind
            grams = [emit_gram(0), emit_gram(1), emit_gram(2)]
            for ir in range(_NT):
                emit_exp(ir, grams[ir])
                if ir + 3 < _NT:
                    grams.append(emit_gram(ir + 3))
                if ir >= 1:
                    emit_pdirect(ir - 1)
                    emit_drain(ir - 1)
            vr = v_out[:].rearrange("p (a b) -> p a b", a=16)
            nc.sync.dma_start(out=vr[:, 0:12, :], in_=Psb[:, 0:12, :])
            emit_pdirect(_NT - 1)
            emit_drain(_NT - 1)
            nc.sync.dma_start(out=vr[:, 12:16, :], in_=Psb[:, 12:16, :])

    nc.compile()
    return nc


def _get_nc():
    global _NC
    if _NC is None:
        _NC = _build_kernel()
    return _NC


def finalize(raw, var, particles):
    """Host-side tail in f64: raw [128, 144] = per-chunk [S_c | D_c]
    pairs (column-scaled by e^{+r_j/2}), var [4], particles [1024, 4]
    -> bandwidth [4]."""
    raw = raw.astype(np.float64).reshape(_P, 16, _NM)
    var = var.astype(np.float64).reshape(_D)
    p = particles.astype(np.float64)
    pilot2 = (_FACT * _FACT) * var

    Vu = np.zeros((_NM, _NM))
    Vd = np.zeros((_NM, _NM))
    for c in range(_NT):
        # chunk c, partition q <-> particle 8q + c
        pc = p[c::_NT]  # [128, 4]
        Mp = np.concatenate([np.ones((_P, 1)), pc, pc * pc], axis=1)
        r = (pc * pc / pilot2).sum(axis=1)
        MX = Mp * np.exp(-0.5 * r)[:, None]
        Vu += MX.T @ raw[:, 2 * c, :]
        Vd += MX.T @ raw[:, 2 * c + 1, :]
    V = Vu + Vu.T - Vd

    d = np.arange(_D)
    s2 = (
        (V[0, 5 + d] + V[5 + d, 0] - 2.0 * V[1 + d, 1 + d]) / pilot2 - V[0, 0]
    ) * _INV_SQRT_2PI
    denom = _N * (_N - 1)
    I2 = s2 / pilot2**2.5 / denom
    J1 = _RK / I2
    base = J1 / _N
    return (np.sign(base) * np.abs(base) ** 0.2).astype(np.float32)


def kernel(particles, weights=None, **_unused):
    from concourse.bass_utils import run_bass_kernel_spmd

    particles = np.ascontiguousarray(np.asarray(particles), dtype=np.float32)
    assert particles.shape == (_B, _N, _D), particles.shape

    nc = _get_nc()
    in_maps = [{"p": particles[c]} for c in range(_B)]
    res = run_bass_kernel_spmd(nc, in_maps, list(range(_B)))

    out = np.empty((_B, _D), np.float32)
    for c in range(_B):
        out[c] = finalize(
            res.results[c]["vout"], res.results[c]["varout"], particles[c]
        )
    return out


# revision 17
# speedup vs baseline: 1.0589x; 1.0073x over previous
"""Trainium2 Bass kernel for nn_BandwidthPredictorNNHall.

Math: for each batch b (8 of them, one per NeuronCore) with particles
x [n=1024, d=4]:
    pilot_d = 1.0592 * std(x_d, ddof=1) * n^(-1/8)
    q = x / pilot,   K_ij = exp(-0.5 * |q_i - q_j|^2)
    s2_d = sum_ij K_ij ((q_jd - q_id)^2 - 1)
    s3 terms are exactly 0 by antisymmetry (treated as 0; fp noise in the
    reference, |bw2/bw1| ~ 6e-9).
With Mp = [1, p_1..p_4, p_1^2..p_4^2] (n x 9, RAW particle units), every sum
needed for s2 is an entry of V = Mp^T K Mp:
    s2_d = ((V[0,5+d] + V[5+d,0] - 2 V[1+d,1+d]) / pilot_d^2 - V[0,0]) / sqrt(2pi)

K-symmetry splits V = Vu + Vu^T - Vd where Vu sums the upper block
triangle (128-row blocks, diagonal blocks whole) and Vd the diagonal
blocks only.  The device computes, per 128-column chunk c, the [128, 9]
partial sums
    S_c[p, m] = sum_{i in upper tiles} K''_{i, j(c,p)} Mp[i, m]
    D_c[p, m] = same but diagonal tile only
where K'' = exp(G - r_i/2) = K * e^{+r_j/2} (the column scale is removed
on the host), and ships the raw [128, 16, 9] block plus var to the host,
which applies MX = Mp * e^{-r/2} and the remaining ~10k-flop reduction in
f64.

Device pipeline per core (latency-driven; ScalarE's exps are the floor):
  - One input DMA in a partition-contiguous layout (partition p holds the
    8 consecutive particles 8p..8p+7 as "tiles" c=0..7: 128 descriptors of
    128B).  All downstream math is permutation-invariant under the
    enumeration j = c*128 + p used consistently on both sides of K.
  - Stats run on the PE in BOTH orientations (row [1,4] for the exp-bias
    broadcast chain, col [4,1] for the per-partition Q scaling) so neither
    needs a transpose of the other.
  - QTr (feature-major f32r) via 8 PE transposes packed as two [4,512]
    PSUM quads (first transpose per quad start=True marks the 2KB
    zero-region, the rest start=False zero-on-touch), drained by two
    512-wide ScalarE copies while DVE runs the var chains.
  - K'' row tiles, UPPER TRIANGLE ONLY: tile ir covers columns
    j >= 128*ir (4608 exp columns instead of 8192), one exp per tile with
    per-partition bias -r_i/2.
  - P-stage contracts over i on the PE with K'' chunks as the stationary
    operand: accT[:, 2jb, :] += KT_chunk(lhsT) @ Mp_tile (~15ns each:
    9-column outputs; weight loads are free), diag-only twin at 2jb+1.
    start=False everywhere + one upfront memset: a start=True would mark
    the whole shared 2KB PSUM bank pending-zero and wipe the co-resident
    accumulators.
  - Chunk c's accumulation finishes at iteration c: one [128,2,9] DVE
    copy per chunk drains it to SBUF in-stream.  After the last tile only
    chunk 7's drain remains, then a single output DMA.
"""

import sys

sys.path.insert(0, "/opt/trn_rl_repo")

import numpy as np

_B, _N, _D = 8, 1024, 4
_P = 128
_NT = _N // _P  # 8 column/row tiles
_NM = 1 + 2 * _D  # 9 basis columns: [1, p, p^2]
_INV_SQRT_2PI = 1.0 / np.sqrt(2.0 * np.pi)
_RK = 0.282095
_FACT = 1.0592 * float(_N) ** (-1.0 / (4 + _D))

_NC = None  # compiled Bass module cache


def _build_kernel():
    import concourse.bass as bass  # noqa: F401
    import concourse.tile as tile
    from concourse import bacc, mybir
    from concourse.masks import make_identity

    f32 = mybir.dt.float32
    fr = mybir.dt.float32r
    Act = mybir.ActivationFunctionType
    Alu = mybir.AluOpType
    Ax = mybir.AxisListType

    nc = bacc.Bacc("TRN2", target_bir_lowering=False, debug=False, num_devices=_B)
    p_in = nc.dram_tensor("p", [_N, _D], f32, kind="ExternalInput")
    v_out = nc.dram_tensor("vout", [_P, 16 * _NM], f32, kind="ExternalOutput")
    var_out = nc.dram_tensor("varout", [1, _D], f32, kind="ExternalOutput")

    with tile.TileContext(nc) as tc:
        with (
            tc.tile_pool(name="singles", bufs=1) as singles,
            tc.tile_pool(name="psE", bufs=1, space="PSUM") as psE,
            tc.tile_pool(name="psA", bufs=1, space="PSUM") as psA,
            tc.tile_pool(name="psG", bufs=3, space="PSUM") as psG,
        ):
            # ---- input DMA first in SP program order (data-ready gates all)
            mstatall = singles.tile([_P, _NT, _D], f32, tag="mstatall")
            nc.sync.dma_start(
                out=mstatall, in_=p_in[:].rearrange("(i c) d -> i c d", i=_P)
            )

            # dummy Exp so the activation-table load runs during the DMA wait
            warm = singles.tile([1, 1], f32, tag="warm")

            ident128 = singles.tile([_P, _P], f32, tag="identf")
            make_identity(nc, ident128)
            ones128 = singles.tile([_P, 1], f32, tag="ones128")
            nc.gpsimd.memset(ones128, 1.0)
            onesN = singles.tile([_P, 1], f32, tag="onesN")
            nc.gpsimd.memset(onesN, 1.0 / float(_N) ** 0.5)
            nc.scalar.activation(out=warm, in_=ident128[0:1, 0:1], func=Act.Exp)

            msqall = singles.tile([_P, _NT, _D], f32, tag="msqall")
            nc.vector.tensor_mul(msqall, mstatall, mstatall)

            # ---- stats on the PE, both orientations, one PSUM bank:
            #  row sums at early[0:1, 4:8] (p) and [0:1, 8:12] (p^2)
            #  col sums at early[0:4, 12:13] (p) and [0:4, 13:14] (p^2)
            early = psE.tile([_P, 16], f32, tag="early")
            for c in range(_NT):
                nc.tensor.matmul(
                    early[0:1, 4:8], lhsT=onesN, rhs=mstatall[:, c, :],
                    start=(c == 0), stop=(c == _NT - 1), skip_group_check=True,
                )
            for c in range(_NT):
                nc.tensor.matmul(
                    early[0:4, 12:13], lhsT=mstatall[:, c, :], rhs=onesN,
                    start=(c == 0), stop=(c == _NT - 1), skip_group_check=True,
                )
            for c in range(_NT):
                nc.tensor.matmul(
                    early[0:1, 8:12], lhsT=ones128, rhs=msqall[:, c, :],
                    start=(c == 0), stop=(c == _NT - 1), skip_group_check=True,
                )
            for c in range(_NT):
                nc.tensor.matmul(
                    early[0:4, 13:14], lhsT=msqall[:, c, :], rhs=ones128,
                    start=(c == 0), stop=(c == _NT - 1), skip_group_check=True,
                )

            # ---- 8 PE transposes -> QTr, packed as two [4,512] quads in
            # rotating psG slots (dead before Gram needs the slots back),
            # each drained by one 512-wide ScalarE copy
            QTr = singles.tile([_D, _N], fr, tag="qtr")
            for q in range(2):
                ps_q = psG.tile([_D, 4 * _P], f32, tag="psg", name=f"psq{q}")
                for k in range(4):
                    c = q * 4 + k
                    nc.tensor.matmul(
                        ps_q[:, k * _P : (k + 1) * _P],
                        lhsT=mstatall[:, c, :], rhs=ident128,
                        is_transpose=True, start=(k == 0), stop=True,
                        skip_group_check=True,
                    )
                cs = slice(q * 4 * _P, (q + 1) * 4 * _P)
                nc.scalar.activation(out=QTr[:, cs], in_=ps_q, func=Act.Copy)

            # ---- var chains on DVE (col form first: it gates the Gram
            # lhsT scale); stats copies from PSUM on DVE
            svr = singles.tile([1, 8], f32, tag="svr")
            nc.vector.tensor_copy(svr, early[0:1, 4:12])
            svc = singles.tile([_D, 2], f32, tag="svc")
            nc.vector.tensor_copy(svc, early[0:4, 12:14])

            den_c = singles.tile([_D, 1], f32, tag="den_c")
            nc.vector.tensor_mul(den_c, svc[:, 0:1], svc[:, 0:1])
            nc.vector.tensor_sub(den_c, svc[:, 1:2], den_c)
            denf_c = singles.tile([_D, 1], f32, tag="denf_c")
            nc.vector.tensor_scalar_mul(denf_c, den_c, _FACT * _FACT / (_N - 1))
            phcol = singles.tile([_D, 1], f32, tag="phcol")
            nc.vector.reciprocal(phcol, denf_c)  # 1/pilot^2

            den_r = singles.tile([1, _D], f32, tag="den_r")
            nc.vector.tensor_mul(den_r, svr[:, 0:4], svr[:, 0:4])
            nc.vector.tensor_sub(den_r, svr[:, 4:8], den_r)
            denf_r = singles.tile([1, _D], f32, tag="denf_r")
            nc.vector.tensor_scalar_mul(denf_r, den_r, 2.0 * _FACT * _FACT / (_N - 1))
            ph_row = singles.tile([1, _D], f32, tag="ph_row")
            nc.vector.reciprocal(ph_row, denf_r)  # 0.5/pilot^2
            var_t = singles.tile([1, _D], f32, tag="var_t")
            nc.gpsimd.tensor_scalar_mul(var_t, den_r, 1.0 / (_N - 1))
            nc.sync.dma_start(out=var_out[:], in_=var_t)

            # bc_sb[128, 4] = ph_row broadcast to all partitions (gpsimd)
            bc_sb = singles.tile([_P, _D], f32, tag="bc_sb")
            nc.gpsimd.partition_broadcast(bc_sb, ph_row)

            # ---- exp-bias chain and Gram lhsT scales, interleaved so
            # tile 0 completes first: scr/qs on gpsimd, reduces on DVE
            qs_t = singles.tile([_D, _NT, _P], fr, tag="qs_t")
            nhall = singles.tile([_P, _NT], f32, tag="nhall")
            scr = singles.tile([_P, _NT, _D], f32, tag="scr")
            for c in range(_NT):
                nc.gpsimd.tensor_mul(scr[:, c, :], msqall[:, c, :], bc_sb)
            for c in range(_NT):
                cs = slice(c * _P, (c + 1) * _P)
                nc.vector.tensor_scalar_mul(qs_t[:, c, :], QTr[:, cs], phcol)
                nc.vector.tensor_reduce(
                    out=nhall[:, c : c + 1], in_=scr[:, c, :],
                    axis=Ax.X, op=Alu.add, negate=True,
                )

            # ---- Mp tiles (f32): [1, p, p^2]
            mtall = singles.tile([_P, _NT, _NM], f32, tag="mtall")
            for c in range(_NT):
                nc.gpsimd.tensor_copy(mtall[:, c, 0:1], ones128)
            nc.gpsimd.tensor_copy(mtall[:, :, 1 : 1 + _D], mstatall)
            nc.gpsimd.tensor_copy(mtall[:, :, 1 + _D : _NM], msqall)

            # ---- PSUM accumulators, one bank: chunk c's upper-sum at
            # [:, 2c, :], diag-only twin at [:, 2c+1, :]
            accT = psA.tile([_P, 16, _NM], f32, tag="accT")
            # start=True on any matmul would mark the whole 2KB zero-region
            # pending-zero and wipe co-resident accumulators: zero once,
            # accumulate-only afterwards
            nc.vector.memset(accT, 0.0)

            KT = singles.tile([_P, _NT, _N], f32, tag="kt")
            Psb = singles.tile([_P, 16, _NM], f32, tag="psb")

            def emit_gram(ir):
                psg = psG.tile([_P, _N], f32, tag="psg", name=f"psg{ir}")
                for a, b in ((0, 512), (512, 1024)):
                    nc.tensor.matmul(
                        psg[:, a:b], lhsT=qs_t[:, ir, :], rhs=QTr[:, a:b],
                        start=True, stop=True,
                    )
                return psg

            def emit_exp(ir, psg):
                s = ir * _P
                nc.scalar.activation(
                    out=KT[:, ir, s:_N], in_=psg[:, s:_N],
                    func=Act.Exp, bias=nhall[:, ir : ir + 1],
                )

            def emit_pdirect(ir):
                for jb in range(ir, _NT):
                    nc.tensor.matmul(
                        accT[:, 2 * jb, :],
                        lhsT=KT[:, ir, jb * _P : (jb + 1) * _P],
                        rhs=mtall[:, ir, :],
                        start=False, stop=(ir == jb),
                        skip_group_check=True,
                    )
                nc.tensor.matmul(
                    accT[:, 2 * ir + 1, :],
                    lhsT=KT[:, ir, ir * _P : (ir + 1) * _P],
                    rhs=mtall[:, ir, :],
                    start=False, stop=True,
                    skip_group_check=True,
                )

            def emit_drain(c):
                nc.vector.tensor_copy(
                    Psb[:, 2 * c : 2 * c + 2, :], accT[:, 2 * c : 2 * c + 2, :]
                )

            # ---- main triangle loop: Grams three ahead (psG bufs=3
            # absorbs the exp->WAR latency), P-stage and drains one behind
            grams = [emit_gram(0), emit_gram(1), emit_gram(2)]
            for ir in range(_NT):
                emit_exp(ir, grams[ir])
                if ir + 3 < _NT:
                    grams.append(emit_gram(ir + 3))
                if ir >= 1:
                    emit_pdirect(ir - 1)
                    emit_drain(ir - 1)
            vr = v_out[:].rearrange("p (a b) -> p a b", a=16)
            nc.sync.dma_start(out=vr[:, 0:12, :], in_=Psb[:, 0:12, :])
            emit_pdirect(_NT - 1)
            emit_drain(_NT - 1)
            nc.sync.dma_start(out=vr[:, 12:16, :], in_=Psb[:, 12:16, :])

    nc.compile()
    return nc


def _get_nc():
    global _NC
    if _NC is None:
        _NC = _build_kernel()
    return _NC


def finalize(raw, var, particles):
    """Host-side tail in f64: raw [128, 144] = per-chunk [S_c | D_c]
    pairs (column-scaled by e^{+r_j/2}), var [4], particles [1024, 4]
    -> bandwidth [4]."""
    raw = raw.astype(np.float64).reshape(_P, 16, _NM)
    var = var.astype(np.float64).reshape(_D)
    p = particles.astype(np.float64)
    pilot2 = (_FACT * _FACT) * var

    Vu = np.zeros((_NM, _NM))
    Vd = np.zeros((_NM, _NM))
    for c in range(_NT):
        # chunk c, partition q <-> particle 8q + c
        pc = p[c::_NT]  # [128, 4]
        Mp = np.concatenate([np.ones((_P, 1)), pc, pc * pc], axis=1)
        r = (pc * pc / pilot2).sum(axis=1)
        MX = Mp * np.exp(-0.5 * r)[:, None]
        Vu += MX.T @ raw[:, 2 * c, :]
        Vd += MX.T @ raw[:, 2 * c + 1, :]
    V = Vu + Vu.T - Vd

    d = np.arange(_D)
    s2 = (
        (V[0, 5 + d] + V[5 + d, 0] - 2.0 * V[1 + d, 1 + d]) / pilot2 - V[0, 0]
    ) * _INV_SQRT_2PI
    denom = _N * (_N - 1)
    I2 = s2 / pilot2**2.5 / denom
    J1 = _RK / I2
    base = J1 / _N
    return (np.sign(base) * np.abs(base) ** 0.2).astype(np.float32)


def kernel(particles, weights=None, **_unused):
    from concourse.bass_utils import run_bass_kernel_spmd

    particles = np.ascontiguousarray(np.asarray(particles), dtype=np.float32)
    assert particles.shape == (_B, _N, _D), particles.shape

    nc = _get_nc()
    in_maps = [{"p": particles[c]} for c in range(_B)]
    res = run_bass_kernel_spmd(nc, in_maps, list(range(_B)))

    out = np.empty((_B, _D), np.float32)
    for c in range(_B):
        out[c] = finalize(
            res.results[c]["vout"], res.results[c]["varout"], particles[c]
        )
    return out
